# revision 40
# baseline (speedup 1.0000x reference)
"""Bass/Trainium2 kernel for nn_CGRE_68719477510 (ragged_sequence).

Restructure: scores[i] = X[i] . Constraints[rel(bag(i))] and the classifier
out = bag @ W.T are both projections of X onto small [53, 2070] matrices.
So one device pass computes Y = [Constraints; W] @ X.T  ([106, N]) — the only
traffic proportional to X (543 MB). The segment softmax + weighted sum then
operate on the projected [N, 53] rows (P = X @ W.T), never touching X again:
    out[bag] = sum_i softmax_i(S) * P[i]  ==  (sum_i w_i X_i) @ W.T
Sharding: split sentences N=65536 into 8 contiguous chunks of 8192 (one per
core); replicate the small combined weight. The ragged segment ops run on
host over the tiny [N, 53] projection.

Device matmul precision (VARIANT="f16"): X, weights and output are all fp16.
fp16's 10-bit mantissa gives ~4x lower score error than bf16, so a single
matmul pass suffices (measured downstream Frobenius rel err 1.7e-3 vs the
2e-2 gate) at HALF the HBM traffic of the previous bf16 hi/lo pair scheme:
~34 MB X in + ~2 MB Y out per core, streamed at the ~358 GB/s per-core HBM
limit. Older variants ("bf16split", "f32r") are kept for reference.
"""

import sys

sys.path.insert(0, "/opt/trn_rl_repo")

import numpy as np

N_SENT = 65536
D_FEAT = 2070
N_REL = 53
N_CORES = 8
N_PER_CORE = N_SENT // N_CORES  # 8192
M_OUT = 2 * N_REL  # 106 rows: [Constraints; W]

KC = 128                      # contraction chunk (partition dim)
N_SUPER = 4096                # sentences per supergroup (fills all 8 PSUM banks)
MM_N = 512                    # moving free dim per matmul (one PSUM bank)
N_KCHUNKS = (D_FEAT + KC - 1) // KC  # 17 (16x128 + 22)

VARIANT = "f16"               # "f16" | "bf16split" | "f32r"

_CACHE = {}


def _build_f16():
    """Single-pass fp16 kernel: Y = [C; W] @ X.T with X, weights, output all fp16.

    fp16 (10-bit mantissa) input rounding gives ~4x lower score error than
    bf16, so one matmul pass suffices (measured downstream Frobenius rel err
    1.7e-3 vs the 2e-2 gate on the real inputs). DMA traffic halves vs the
    bf16 hi/lo pair: 34 MB X + 0.45 MB weights in, 1.7 MB out per core.

    Structure: 2 "pages" of 4096 columns; per page, 17 k-chunk DMAs of
    [128, 4096] fp16 (1 MB each, alternating the two HWDGE rings), each
    consumed immediately by 8 matmuls (4 strips x 2 N=512 slices)
    accumulating into four 2-bank PSUM strips. X tiles free right after use
    -> deep DMA prefetch (bufs=6). Strip-granular DVE casts (f32->f16)
    pipeline behind the page's last matmuls; each [128, 2048] output half
    stores on its own ring as soon as its two casts land, overlapping the
    remaining casts. Stores are padded to 128 partitions: a 106-partition
    SBUF->HBM DMA degenerates to one SDMA engine (~31 GB/s vs ~394).
    Measured ~112.4us (fast mode) / ~125us (slow mode; bimodality is HBM
    stack contention phase with the paired NeuronCore, not controllable).
    """
    import concourse.mybir as mybir
    from concourse import bacc
    from concourse.tile import TileContext

    F16 = mybir.dt.float16
    F32 = mybir.dt.float32

    nc = bacc.Bacc("TRN2", target_bir_lowering=False, debug=True)
    # page-blocked X: entry (pg, k) is one fully-contiguous 1 MB HBM read
    # (vs 128 x 8KB stride-16KB in the flat [D_FEAT, N] layout)
    xf = nc.dram_tensor(
        "xf",
        [(N_PER_CORE // 4096) * N_KCHUNKS, KC, 4096],
        F16,
        kind="ExternalInput",
    )
    cwt = nc.dram_tensor("cwt", [KC, N_KCHUNKS * M_OUT], F16, kind="ExternalInput")

    PAGE = 4096                       # columns per page (fills all 8 PSUM banks)
    n_pages = N_PER_CORE // PAGE      # 2
    SUP = 2048                        # columns per psum tile (4 banks)
    # 128 partitions (rows 106-127 garbage): a 106-partition SBUF->HBM DMA
    # degenerates to 106 serial 4KB descriptors on ONE SDMA engine (~31 GB/s);
    # [128, 4096] fans 8KB descriptors over all 16 engines (~394 GB/s).
    yt = nc.dram_tensor(
        "yt", [n_pages * 2, KC, PAGE // 2], F16, kind="ExternalOutput"
    )

    with TileContext(nc) as tc:
        STRIP = 1024                  # psum strip: 2 banks; 4 strips = one page
        with (
            tc.tile_pool(name="w", bufs=1) as wpool,
            tc.tile_pool(name="x", bufs=6) as xpool,
            tc.tile_pool(name="out", bufs=4) as opool,
            tc.tile_pool(name="psum", bufs=4, space="PSUM") as ppool,
        ):
            wtile = wpool.tile([KC, N_KCHUNKS * M_OUT], F16, tag="w")
            nc.sync.dma_start(out=wtile[:, :], in_=cwt[:, :])

            korder = list(range(N_KCHUNKS))
            for pg in range(n_pages):
                c0 = pg * PAGE
                psums = []
                for _pi in range(4):
                    ps_t = ppool.tile([M_OUT, STRIP], F32, tag="ps",
                                      name=f"ps{_pi}")
                    psums.append(ps_t)
                for ki, k in enumerate(korder):
                    kp = min(KC, D_FEAT - k * KC)
                    xt = xpool.tile([KC, PAGE], F16, tag="x")
                    eng = nc.sync if ki % 2 == 0 else nc.scalar
                    eng.dma_start(
                        out=xt[:kp], in_=xf[pg * N_KCHUNKS + k][:kp, :]
                    )
                    ws = slice(k * M_OUT, (k + 1) * M_OUT)
                    for st in range(4):
                        for s in range(2):
                            off = st * STRIP + s * MM_N
                            nc.tensor.matmul(
                                psums[st][:, s * MM_N : (s + 1) * MM_N],
                                wtile[:kp, ws],
                                xt[:kp, off : off + MM_N],
                                start=(ki == 0),
                                stop=(ki == N_KCHUNKS - 1),
                            )
                # strip casts pipeline behind the page's last matmuls; each
                # [128, 2048] half fires on its own HWDGE ring right after its
                # two casts, so the final store overlaps the preceding casts
                # (HWDGE, not gpsimd: SWDGE quiesce at exit costs ~15us)
                for h in range(2):
                    out_t = opool.tile([KC, PAGE // 2], F16, tag="out")
                    for j, st in enumerate(range(2 * h, 2 * h + 2)):
                        nc.vector.tensor_copy(
                            out=out_t[:M_OUT, j * STRIP : (j + 1) * STRIP],
                            in_=psums[st][:, :],
                        )
                    eng = nc.sync if (pg + h) % 2 == 0 else nc.scalar
                    eng.dma_start(out=yt[pg * 2 + h], in_=out_t[:, :])

    nc.compile()
    return nc, SUP


def _build_f32r():
    import concourse.mybir as mybir
    from concourse import bacc
    from concourse.tile import TileContext

    DT = mybir.dt.float32r  # fp32 bits, full-rate PE streaming mode
    F32 = mybir.dt.float32

    nc = bacc.Bacc("TRN2", target_bir_lowering=False, debug=True)
    xt = nc.dram_tensor("xt", [D_FEAT, N_PER_CORE], DT, kind="ExternalInput")
    # weights packed on host: wpack[p, k*106+m] = CW[m, 128k+p] (zero-padded)
    cwt = nc.dram_tensor("cwt", [KC, N_KCHUNKS * M_OUT], DT, kind="ExternalInput")

    n_supers = N_PER_CORE // N_SUPER  # 2
    subs = N_SUPER // MM_N            # 8 (one PSUM bank each)
    XSPLIT = 1024                     # columns per x dma_start / tile
    nsplit = N_SUPER // XSPLIT        # 4

    # output in [block, 128, XSPLIT] layout: contiguous per-DMA, 128 partitions
    # (rows 106..127 are padding garbage; host slices them off)
    yt = nc.dram_tensor(
        "yt", [n_supers * nsplit, KC, XSPLIT], F32, kind="ExternalOutput"
    )

    with TileContext(nc) as tc:
        with (
            tc.tile_pool(name="w", bufs=1) as wpool,
            tc.tile_pool(name="x", bufs=6 * nsplit) as xpool,
            tc.tile_pool(name="out", bufs=8) as opool,
            tc.tile_pool(name="psum", bufs=1, space="PSUM") as ppool,
        ):
            wtile = wpool.tile([KC, N_KCHUNKS * M_OUT], DT, tag="w")
            nc.sync.dma_start(out=wtile[:, :], in_=cwt[:, :])

            for sp in range(n_supers):
                c0 = sp * N_SUPER
                psum = ppool.tile([M_OUT, N_SUPER], F32, tag="ps")
                for k in range(N_KCHUNKS):
                    k0 = k * KC
                    kp = min(KC, D_FEAT - k0)
                    xts = []
                    for j in range(nsplit):
                        xtile = xpool.tile([KC, XSPLIT], DT, tag="x")
                        eng = nc.sync if j % 2 == 0 else nc.scalar
                        eng.dma_start(
                            out=xtile[:kp],
                            in_=xt[
                                k0 : k0 + kp,
                                c0 + j * XSPLIT : c0 + (j + 1) * XSPLIT,
                            ],
                        )
                        xts.append(xtile)
                    for s in range(subs):
                        xt_j = xts[(s * MM_N) // XSPLIT]
                        off = (s * MM_N) % XSPLIT
                        nc.tensor.matmul(
                            psum[:, s * MM_N : (s + 1) * MM_N],
                            wtile[:kp, k * M_OUT : (k + 1) * M_OUT],
                            xt_j[:kp, off : off + MM_N],
                            start=(k == 0),
                            stop=(k == N_KCHUNKS - 1),
                        )
                for j in range(nsplit):
                    out_t = opool.tile([KC, XSPLIT], F32, tag="out")
                    nc.vector.tensor_copy(
                        out=out_t[:M_OUT, :],
                        in_=psum[:, j * XSPLIT : (j + 1) * XSPLIT],
                    )
                    nc.sync.dma_start(out=yt[sp * nsplit + j], in_=out_t[:, :])

    nc.compile()
    return nc, 1024


def _build_bf16split():
    import concourse.mybir as mybir
    from concourse import bacc
    from concourse.tile import TileContext

    BF = mybir.dt.bfloat16
    F32 = mybir.dt.float32

    nc = bacc.Bacc("TRN2", target_bir_lowering=False, debug=True)
    xh = nc.dram_tensor("xh", [D_FEAT, N_PER_CORE], BF, kind="ExternalInput")
    xl = nc.dram_tensor("xl", [D_FEAT, N_PER_CORE], BF, kind="ExternalInput")
    cwh = nc.dram_tensor("cwh", [KC, N_KCHUNKS * M_OUT], BF, kind="ExternalInput")
    cwl = nc.dram_tensor("cwl", [KC, N_KCHUNKS * M_OUT], BF, kind="ExternalInput")

    n_super = 2048                    # half PSUM per supergroup -> ping-pong
    n_supers = N_PER_CORE // n_super  # 4
    subs = n_super // MM_N            # 4
    XSPLIT = 2048                     # bf16: 4KB descriptors at 2048 cols
    nsplit = n_super // XSPLIT        # 1

    yt = nc.dram_tensor(
        "yt", [n_supers * nsplit, KC, XSPLIT], F32, kind="ExternalOutput"
    )

    with TileContext(nc) as tc:
        with (
            tc.tile_pool(name="w", bufs=1) as wpool,
            tc.tile_pool(name="x", bufs=7 * 2 * nsplit) as xpool,
            tc.tile_pool(name="out", bufs=4) as opool,
            tc.tile_pool(name="psum", bufs=2, space="PSUM") as ppool,
        ):
            wh = wpool.tile([KC, N_KCHUNKS * M_OUT], BF, tag="wh")
            nc.sync.dma_start(out=wh[:, :], in_=cwh[:, :])
            wl = wpool.tile([KC, N_KCHUNKS * M_OUT], BF, tag="wl")
            nc.scalar.dma_start(out=wl[:, :], in_=cwl[:, :])

            for sp in range(n_supers):
                c0 = sp * n_super
                psum = ppool.tile([M_OUT, n_super], F32, tag="ps")
                for k in range(N_KCHUNKS):
                    k0 = k * KC
                    kp = min(KC, D_FEAT - k0)
                    xh_ts, xl_ts = [], []
                    for j in range(nsplit):
                        cs = slice(c0 + j * XSPLIT, c0 + (j + 1) * XSPLIT)
                        th = xpool.tile([KC, XSPLIT], BF, tag="x")
                        eng = nc.sync if j % 2 == 0 else nc.scalar
                        eng.dma_start(out=th[:kp], in_=xh[k0 : k0 + kp, cs])
                        xh_ts.append(th)
                        tl = xpool.tile([KC, XSPLIT], BF, tag="x")
                        eng = nc.scalar if j % 2 == 0 else nc.sync
                        eng.dma_start(out=tl[:kp], in_=xl[k0 : k0 + kp, cs])
                        xl_ts.append(tl)
                    ws = slice(k * M_OUT, (k + 1) * M_OUT)
                    for s in range(subs):
                        j = (s * MM_N) // XSPLIT
                        off = (s * MM_N) % XSPLIT
                        for wt, xs, st, sp_ in (
                            (wh, xh_ts, k == 0, False),
                            (wl, xh_ts, False, False),
                            (wh, xl_ts, False, k == N_KCHUNKS - 1),
                        ):
                            nc.tensor.matmul(
                                psum[:, s * MM_N : (s + 1) * MM_N],
                                wt[:kp, ws],
                                xs[j][:kp, off : off + MM_N],
                                start=st,
                                stop=sp_,
                            )
                for j in range(nsplit):
                    out_t = opool.tile([KC, XSPLIT], F32, tag="out")
                    nc.vector.tensor_copy(
                        out=out_t[:M_OUT, :],
                        in_=psum[:, j * XSPLIT : (j + 1) * XSPLIT],
                    )
                    eng = nc.sync if (sp * nsplit + j) % 2 == 0 else nc.scalar
                    eng.dma_start(out=yt[sp * nsplit + j], in_=out_t[:, :])

    nc.compile()
    return nc, XSPLIT


def _build(variant=None):
    variant = variant or VARIANT
    if variant not in _CACHE:
        builders = {
            "f16": _build_f16,
            "bf16split": _build_bf16split,
            "f32r": _build_f32r,
        }
        _CACHE[variant] = builders[variant]()
    return _CACHE[variant]


def _pack_weights(CWT, dtype=np.float32):
    """CWT [D_FEAT, 106] -> [128, 17*106] with wpack[p, k*106+m] = CWT[128k+p, m]."""
    pad = N_KCHUNKS * KC - D_FEAT
    cw = np.concatenate(
        [CWT.astype(np.float32), np.zeros((pad, M_OUT), dtype=np.float32)], axis=0
    )  # [2176, 106]
    return np.ascontiguousarray(
        cw.reshape(N_KCHUNKS, KC, M_OUT).transpose(1, 0, 2).reshape(KC, -1)
    ).astype(dtype)


def _unpack_yt(res, xsplit, variant=None):
    variant = variant or VARIANT
    if variant == "f16":
        return np.concatenate(
            [
                res.results[c]["yt"][:, :M_OUT, :]
                .astype(np.float32)
                .transpose(1, 0, 2)
                .reshape(M_OUT, N_PER_CORE)
                for c in range(N_CORES)
            ],
            axis=1,
        )
    return np.concatenate(
        [
            res.results[c]["yt"][:, :M_OUT, :]
            .transpose(1, 0, 2)
            .reshape(M_OUT, N_PER_CORE)
            for c in range(N_CORES)
        ],
        axis=1,
    )


def _ensure_ntff_hook():
    """bass_utils' trace path hard-imports antenv.axon_hooks, which this image
    lacks; shim it so a BASS_TRACE env var (or trace=True) can't crash."""
    import types

    try:
        from antenv.axon_hooks import get_axon_ntff_profile_hook  # noqa: F401

        return
    except ImportError:
        pass
    try:
        import antenv
        from trn_agent_boot.trn_boot import _ntff_profile_via_ctypes

        hook = _ntff_profile_via_ctypes("/opt/axon/libaxon_pjrt.so")
    except Exception:
        antenv, hook = None, None
    mod = types.ModuleType("antenv.axon_hooks")
    _h = [hook]
    mod.set_axon_ntff_profile_hook = lambda h: _h.__setitem__(0, h)
    mod.get_axon_ntff_profile_hook = lambda: _h[0]
    sys.modules["antenv.axon_hooks"] = mod
    if antenv is not None:
        antenv.axon_hooks = mod


def _run_device(XT, CWT, trace=False, variant=None):
    """XT [D_FEAT, N_SENT] f32, CWT [D_FEAT, 106] f32 -> YT [106, N_SENT] f32."""
    _ensure_ntff_hook()
    from concourse.bass_utils import run_bass_kernel_spmd

    variant = variant or VARIANT
    nc, xsplit = _build(variant)

    if variant == "f16":
        XF = XT.astype(np.float16)
        wpack = _pack_weights(CWT, np.float16)
        n_pg = N_PER_CORE // 4096
        in_maps = []
        for c in range(N_CORES):
            Xc = XF[:, c * N_PER_CORE : (c + 1) * N_PER_CORE]
            xfb = np.zeros((n_pg * N_KCHUNKS, KC, 4096), dtype=np.float16)
            for pg in range(n_pg):
                for k in range(N_KCHUNKS):
                    kp = min(KC, D_FEAT - k * KC)
                    xfb[pg * N_KCHUNKS + k, :kp] = Xc[
                        k * KC : k * KC + kp, pg * 4096 : (pg + 1) * 4096
                    ]
            in_maps.append({"xf": xfb, "cwt": wpack})
    elif variant == "f32r":
        wpack = _pack_weights(CWT)
        in_maps = [
            {
                "xt": np.ascontiguousarray(
                    XT[:, c * N_PER_CORE : (c + 1) * N_PER_CORE]
                ),
                "cwt": wpack,
            }
            for c in range(N_CORES)
        ]
    else:
        import ml_dtypes

        bf16 = ml_dtypes.bfloat16
        XH = XT.astype(bf16)
        XL = (XT - XH.astype(np.float32)).astype(bf16)
        CWH = CWT.astype(np.float32).astype(bf16).astype(np.float32)
        CWL = CWT.astype(np.float32) - CWH
        wh = _pack_weights(CWH, bf16)
        wl = _pack_weights(CWL, bf16)
        in_maps = [
            {
                "xh": np.ascontiguousarray(
                    XH[:, c * N_PER_CORE : (c + 1) * N_PER_CORE]
                ),
                "xl": np.ascontiguousarray(
                    XL[:, c * N_PER_CORE : (c + 1) * N_PER_CORE]
                ),
                "cwh": wh,
                "cwl": wl,
            }
            for c in range(N_CORES)
        ]

    res = run_bass_kernel_spmd(nc, in_maps, list(range(N_CORES)), trace=trace)
    return _unpack_yt(res, xsplit, variant), res


def kernel(X, Constraints, W, b, X_Scope, X_Rel, _trace=False, _res_out=None):
    X = np.asarray(X)
    Constraints = np.asarray(Constraints)
    W = np.asarray(W)
    b = np.asarray(b)
    X_Scope = np.asarray(X_Scope)
    X_Rel = np.asarray(X_Rel)

    N, D = X.shape
    B = X_Scope.shape[0]
    R = Constraints.shape[0]
    assert (N, D, R) == (N_SENT, D_FEAT, N_REL), (N, D, R)

    XT = np.ascontiguousarray(X.T)
    CWT = np.ascontiguousarray(
        np.concatenate([Constraints, W], axis=0).T.astype(np.float32)
    )

    YT, res = _run_device(XT, CWT, trace=_trace)
    if _res_out is not None:
        _res_out.append(res)

    S_all = YT[:N_REL]          # [53, N] scores for every relation
    P = YT[N_REL:]              # [53, N] per-sentence classifier projections

    # host downstream on [N, 53]-sized data (mirrors reference semantics)
    starts = X_Scope[:, 0].astype(np.int64)
    seg = np.searchsorted(starts, np.arange(N, dtype=np.int64), side="right") - 1
    rel = np.asarray(X_Rel)[seg]  # wraps for seg == -1, same as jnp
    s = S_all[rel, np.arange(N)].astype(np.float64)

    valid = seg >= 0
    segv = seg[valid]
    m = np.full(B, -np.inf)
    np.maximum.at(m, segv, s[valid])
    e = np.exp(s - np.where(valid, m[np.clip(seg, 0, B - 1)], np.inf))
    e = np.where(valid, e, 0.0)
    z = np.bincount(segv, weights=e[valid], minlength=B)
    zsafe = np.where(z == 0.0, 1.0, z)
    w = e / zsafe[np.clip(seg, 0, B - 1)]

    out = np.empty((B, N_REL), dtype=np.float64)
    Pw = P.astype(np.float64) * w[None, :]
    for j in range(N_REL):
        out[:, j] = np.bincount(segv, weights=Pw[j, valid], minlength=B)
    out += b.astype(np.float64)[None, :]
    return out.astype(np.float32)



# revision 43
# speedup vs baseline: 1.1150x; 1.1150x over previous
"""Bass/Trainium2 kernel for nn_CGRE_68719477510 (ragged_sequence).

Restructure: scores[i] = X[i] . Constraints[rel(bag(i))] and the classifier
out = bag @ W.T are both projections of X onto small [53, 2070] matrices.
So one device pass computes Y = [Constraints; W] @ X.T  ([106, N]) — the only
traffic proportional to X (543 MB). The segment softmax + weighted sum then
operate on the projected [N, 53] rows (P = X @ W.T), never touching X again:
    out[bag] = sum_i softmax_i(S) * P[i]  ==  (sum_i w_i X_i) @ W.T
Sharding: split sentences N=65536 into 8 contiguous chunks of 8192 (one per
core); replicate the small combined weight. The ragged segment ops run on
host over the tiny [N, 53] projection.

Device matmul precision (VARIANT="f16"): X, weights and output are all fp16.
fp16's 10-bit mantissa gives ~4x lower score error than bf16, so a single
matmul pass suffices (measured downstream Frobenius rel err 1.7e-3 vs the
2e-2 gate) at HALF the HBM traffic of the previous bf16 hi/lo pair scheme:
~34 MB X in + ~2 MB Y out per core, streamed at the ~358 GB/s per-core HBM
limit. Older variants ("bf16split", "f32r") are kept for reference.
"""

import sys

sys.path.insert(0, "/opt/trn_rl_repo")

import numpy as np

N_SENT = 65536
D_FEAT = 2070
N_REL = 53
N_CORES = 8
N_PER_CORE = N_SENT // N_CORES  # 8192
M_OUT = 2 * N_REL  # 106 rows: [Constraints; W]

KC = 128                      # contraction chunk (partition dim)
N_SUPER = 4096                # sentences per supergroup (fills all 8 PSUM banks)
MM_N = 512                    # moving free dim per matmul (one PSUM bank)
N_KCHUNKS = (D_FEAT + KC - 1) // KC  # 17 (16x128 + 22)

VARIANT = "f16"               # "f16" | "bf16split" | "f32r"

_CACHE = {}


def _build_f16():
    """Single-pass fp16 kernel: Y = [C; W] @ X.T with X, weights, output all fp16.

    fp16 (10-bit mantissa) input rounding gives ~4x lower score error than
    bf16, so one matmul pass suffices (measured downstream Frobenius rel err
    1.7e-3 vs the 2e-2 gate on the real inputs). DMA traffic halves vs the
    bf16 hi/lo pair: 34 MB X + 0.45 MB weights in, 1.7 MB out per core.

    Structure: 2 "pages" of 4096 columns; per page, 17 k-chunk DMAs of
    [128, 4096] fp16 (1 MB each, alternating the two HWDGE rings), each
    consumed immediately by 8 matmuls (4 strips x 2 N=512 slices)
    accumulating into four 2-bank PSUM strips. X tiles free right after use
    -> deep DMA prefetch (bufs=6). Strip-granular DVE casts (f32->f16)
    pipeline behind the page's last matmuls; each [128, 2048] output half
    stores on its own ring as soon as its two casts land, overlapping the
    remaining casts. Stores are padded to 128 partitions: a 106-partition
    SBUF->HBM DMA degenerates to one SDMA engine (~31 GB/s vs ~394).
    Measured ~112.4us (fast mode) / ~125us (slow mode; bimodality is HBM
    stack contention phase with the paired NeuronCore, not controllable).
    """
    import concourse.mybir as mybir
    from concourse import bacc
    from concourse.tile import TileContext

    F16 = mybir.dt.float16
    F32 = mybir.dt.float32

    nc = bacc.Bacc("TRN2", target_bir_lowering=False, debug=True)
    xf = nc.dram_tensor("xf", [D_FEAT, N_PER_CORE], F16, kind="ExternalInput")
    cwt = nc.dram_tensor("cwt", [KC, N_KCHUNKS * M_OUT], F16, kind="ExternalInput")

    PAGE = 4096                       # columns per page (fills all 8 PSUM banks)
    n_pages = N_PER_CORE // PAGE      # 2
    SUP = 2048                        # columns per psum tile (4 banks)
    # 128 partitions (rows 106-127 garbage): a 106-partition SBUF->HBM DMA
    # degenerates to 106 serial 4KB descriptors on ONE SDMA engine (~31 GB/s);
    # [128, 4096] fans 8KB descriptors over all 16 engines (~394 GB/s).
    yt = nc.dram_tensor(
        "yt", [n_pages * 2, KC, PAGE // 2], F16, kind="ExternalOutput"
    )

    with TileContext(nc) as tc:
        STRIP = 1024                  # psum strip: 2 banks; 4 strips = one page
        with (
            tc.tile_pool(name="w", bufs=1) as wpool,
            tc.tile_pool(name="x", bufs=6) as xpool,
            tc.tile_pool(name="out", bufs=4) as opool,
            tc.tile_pool(name="psum", bufs=4, space="PSUM") as ppool,
        ):
            wtile = wpool.tile([KC, N_KCHUNKS * M_OUT], F16, tag="w")
            nc.sync.dma_start(out=wtile[:, :], in_=cwt[:, :])

            korder = list(range(N_KCHUNKS))
            for pg in range(n_pages):
                c0 = pg * PAGE
                psums = []
                for _pi in range(4):
                    ps_t = ppool.tile([M_OUT, STRIP], F32, tag="ps",
                                      name=f"ps{_pi}")
                    psums.append(ps_t)
                for ki, k in enumerate(korder):
                    k0 = k * KC
                    kp = min(KC, D_FEAT - k0)
                    xt = xpool.tile([KC, PAGE], F16, tag="x")
                    eng = nc.sync if ki % 2 == 0 else nc.scalar
                    eng.dma_start(
                        out=xt[:kp], in_=xf[k0 : k0 + kp, c0 : c0 + PAGE]
                    )
                    ws = slice(k * M_OUT, (k + 1) * M_OUT)
                    for st in range(4):
                        for s in range(2):
                            off = st * STRIP + s * MM_N
                            nc.tensor.matmul(
                                psums[st][:, s * MM_N : (s + 1) * MM_N],
                                wtile[:kp, ws],
                                xt[:kp, off : off + MM_N],
                                start=(ki == 0),
                                stop=(ki == N_KCHUNKS - 1),
                            )
                # strip casts pipeline behind the page's last matmuls; each
                # [128, 2048] half fires on its own HWDGE ring right after its
                # two casts, so the final store overlaps the preceding casts
                # (HWDGE, not gpsimd: SWDGE quiesce at exit costs ~15us)
                for h in range(2):
                    out_t = opool.tile([KC, PAGE // 2], F16, tag="out")
                    for j, st in enumerate(range(2 * h, 2 * h + 2)):
                        nc.vector.tensor_copy(
                            out=out_t[:M_OUT, j * STRIP : (j + 1) * STRIP],
                            in_=psums[st][:, :],
                        )
                    eng = nc.sync if (pg + h) % 2 == 0 else nc.scalar
                    eng.dma_start(out=yt[pg * 2 + h], in_=out_t[:, :])

    nc.compile()
    return nc, SUP


def _build_f32r():
    import concourse.mybir as mybir
    from concourse import bacc
    from concourse.tile import TileContext

    DT = mybir.dt.float32r  # fp32 bits, full-rate PE streaming mode
    F32 = mybir.dt.float32

    nc = bacc.Bacc("TRN2", target_bir_lowering=False, debug=True)
    xt = nc.dram_tensor("xt", [D_FEAT, N_PER_CORE], DT, kind="ExternalInput")
    # weights packed on host: wpack[p, k*106+m] = CW[m, 128k+p] (zero-padded)
    cwt = nc.dram_tensor("cwt", [KC, N_KCHUNKS * M_OUT], DT, kind="ExternalInput")

    n_supers = N_PER_CORE // N_SUPER  # 2
    subs = N_SUPER // MM_N            # 8 (one PSUM bank each)
    XSPLIT = 1024                     # columns per x dma_start / tile
    nsplit = N_SUPER // XSPLIT        # 4

    # output in [block, 128, XSPLIT] layout: contiguous per-DMA, 128 partitions
    # (rows 106..127 are padding garbage; host slices them off)
    yt = nc.dram_tensor(
        "yt", [n_supers * nsplit, KC, XSPLIT], F32, kind="ExternalOutput"
    )

    with TileContext(nc) as tc:
        with (
            tc.tile_pool(name="w", bufs=1) as wpool,
            tc.tile_pool(name="x", bufs=6 * nsplit) as xpool,
            tc.tile_pool(name="out", bufs=8) as opool,
            tc.tile_pool(name="psum", bufs=1, space="PSUM") as ppool,
        ):
            wtile = wpool.tile([KC, N_KCHUNKS * M_OUT], DT, tag="w")
            nc.sync.dma_start(out=wtile[:, :], in_=cwt[:, :])

            for sp in range(n_supers):
                c0 = sp * N_SUPER
                psum = ppool.tile([M_OUT, N_SUPER], F32, tag="ps")
                for k in range(N_KCHUNKS):
                    k0 = k * KC
                    kp = min(KC, D_FEAT - k0)
                    xts = []
                    for j in range(nsplit):
                        xtile = xpool.tile([KC, XSPLIT], DT, tag="x")
                        eng = nc.sync if j % 2 == 0 else nc.scalar
                        eng.dma_start(
                            out=xtile[:kp],
                            in_=xt[
                                k0 : k0 + kp,
                                c0 + j * XSPLIT : c0 + (j + 1) * XSPLIT,
                            ],
                        )
                        xts.append(xtile)
                    for s in range(subs):
                        xt_j = xts[(s * MM_N) // XSPLIT]
                        off = (s * MM_N) % XSPLIT
                        nc.tensor.matmul(
                            psum[:, s * MM_N : (s + 1) * MM_N],
                            wtile[:kp, k * M_OUT : (k + 1) * M_OUT],
                            xt_j[:kp, off : off + MM_N],
                            start=(k == 0),
                            stop=(k == N_KCHUNKS - 1),
                        )
                for j in range(nsplit):
                    out_t = opool.tile([KC, XSPLIT], F32, tag="out")
                    nc.vector.tensor_copy(
                        out=out_t[:M_OUT, :],
                        in_=psum[:, j * XSPLIT : (j + 1) * XSPLIT],
                    )
                    nc.sync.dma_start(out=yt[sp * nsplit + j], in_=out_t[:, :])

    nc.compile()
    return nc, 1024


def _build_bf16split():
    import concourse.mybir as mybir
    from concourse import bacc
    from concourse.tile import TileContext

    BF = mybir.dt.bfloat16
    F32 = mybir.dt.float32

    nc = bacc.Bacc("TRN2", target_bir_lowering=False, debug=True)
    xh = nc.dram_tensor("xh", [D_FEAT, N_PER_CORE], BF, kind="ExternalInput")
    xl = nc.dram_tensor("xl", [D_FEAT, N_PER_CORE], BF, kind="ExternalInput")
    cwh = nc.dram_tensor("cwh", [KC, N_KCHUNKS * M_OUT], BF, kind="ExternalInput")
    cwl = nc.dram_tensor("cwl", [KC, N_KCHUNKS * M_OUT], BF, kind="ExternalInput")

    n_super = 2048                    # half PSUM per supergroup -> ping-pong
    n_supers = N_PER_CORE // n_super  # 4
    subs = n_super // MM_N            # 4
    XSPLIT = 2048                     # bf16: 4KB descriptors at 2048 cols
    nsplit = n_super // XSPLIT        # 1

    yt = nc.dram_tensor(
        "yt", [n_supers * nsplit, KC, XSPLIT], F32, kind="ExternalOutput"
    )

    with TileContext(nc) as tc:
        with (
            tc.tile_pool(name="w", bufs=1) as wpool,
            tc.tile_pool(name="x", bufs=7 * 2 * nsplit) as xpool,
            tc.tile_pool(name="out", bufs=4) as opool,
            tc.tile_pool(name="psum", bufs=2, space="PSUM") as ppool,
        ):
            wh = wpool.tile([KC, N_KCHUNKS * M_OUT], BF, tag="wh")
            nc.sync.dma_start(out=wh[:, :], in_=cwh[:, :])
            wl = wpool.tile([KC, N_KCHUNKS * M_OUT], BF, tag="wl")
            nc.scalar.dma_start(out=wl[:, :], in_=cwl[:, :])

            for sp in range(n_supers):
                c0 = sp * n_super
                psum = ppool.tile([M_OUT, n_super], F32, tag="ps")
                for k in range(N_KCHUNKS):
                    k0 = k * KC
                    kp = min(KC, D_FEAT - k0)
                    xh_ts, xl_ts = [], []
                    for j in range(nsplit):
                        cs = slice(c0 + j * XSPLIT, c0 + (j + 1) * XSPLIT)
                        th = xpool.tile([KC, XSPLIT], BF, tag="x")
                        eng = nc.sync if j % 2 == 0 else nc.scalar
                        eng.dma_start(out=th[:kp], in_=xh[k0 : k0 + kp, cs])
                        xh_ts.append(th)
                        tl = xpool.tile([KC, XSPLIT], BF, tag="x")
                        eng = nc.scalar if j % 2 == 0 else nc.sync
                        eng.dma_start(out=tl[:kp], in_=xl[k0 : k0 + kp, cs])
                        xl_ts.append(tl)
                    ws = slice(k * M_OUT, (k + 1) * M_OUT)
                    for s in range(subs):
                        j = (s * MM_N) // XSPLIT
                        off = (s * MM_N) % XSPLIT
                        for wt, xs, st, sp_ in (
                            (wh, xh_ts, k == 0, False),
                            (wl, xh_ts, False, False),
                            (wh, xl_ts, False, k == N_KCHUNKS - 1),
                        ):
                            nc.tensor.matmul(
                                psum[:, s * MM_N : (s + 1) * MM_N],
                                wt[:kp, ws],
                                xs[j][:kp, off : off + MM_N],
                                start=st,
                                stop=sp_,
                            )
                for j in range(nsplit):
                    out_t = opool.tile([KC, XSPLIT], F32, tag="out")
                    nc.vector.tensor_copy(
                        out=out_t[:M_OUT, :],
                        in_=psum[:, j * XSPLIT : (j + 1) * XSPLIT],
                    )
                    eng = nc.sync if (sp * nsplit + j) % 2 == 0 else nc.scalar
                    eng.dma_start(out=yt[sp * nsplit + j], in_=out_t[:, :])

    nc.compile()
    return nc, XSPLIT


def _build(variant=None):
    variant = variant or VARIANT
    if variant not in _CACHE:
        builders = {
            "f16": _build_f16,
            "bf16split": _build_bf16split,
            "f32r": _build_f32r,
        }
        _CACHE[variant] = builders[variant]()
    return _CACHE[variant]


def _pack_weights(CWT, dtype=np.float32):
    """CWT [D_FEAT, 106] -> [128, 17*106] with wpack[p, k*106+m] = CWT[128k+p, m]."""
    pad = N_KCHUNKS * KC - D_FEAT
    cw = np.concatenate(
        [CWT.astype(np.float32), np.zeros((pad, M_OUT), dtype=np.float32)], axis=0
    )  # [2176, 106]
    return np.ascontiguousarray(
        cw.reshape(N_KCHUNKS, KC, M_OUT).transpose(1, 0, 2).reshape(KC, -1)
    ).astype(dtype)


def _unpack_yt(res, xsplit, variant=None):
    variant = variant or VARIANT
    if variant == "f16":
        return np.concatenate(
            [
                res.results[c]["yt"][:, :M_OUT, :]
                .astype(np.float32)
                .transpose(1, 0, 2)
                .reshape(M_OUT, N_PER_CORE)
                for c in range(N_CORES)
            ],
            axis=1,
        )
    return np.concatenate(
        [
            res.results[c]["yt"][:, :M_OUT, :]
            .transpose(1, 0, 2)
            .reshape(M_OUT, N_PER_CORE)
            for c in range(N_CORES)
        ],
        axis=1,
    )


def _ensure_ntff_hook():
    """bass_utils' trace path hard-imports antenv.axon_hooks, which this image
    lacks; shim it so a BASS_TRACE env var (or trace=True) can't crash."""
    import types

    try:
        from antenv.axon_hooks import get_axon_ntff_profile_hook  # noqa: F401

        return
    except ImportError:
        pass
    try:
        import antenv
        from trn_agent_boot.trn_boot import _ntff_profile_via_ctypes

        hook = _ntff_profile_via_ctypes("/opt/axon/libaxon_pjrt.so")
    except Exception:
        antenv, hook = None, None
    mod = types.ModuleType("antenv.axon_hooks")
    _h = [hook]
    mod.set_axon_ntff_profile_hook = lambda h: _h.__setitem__(0, h)
    mod.get_axon_ntff_profile_hook = lambda: _h[0]
    sys.modules["antenv.axon_hooks"] = mod
    if antenv is not None:
        antenv.axon_hooks = mod


def _run_device(XT, CWT, trace=False, variant=None):
    """XT [D_FEAT, N_SENT] f32, CWT [D_FEAT, 106] f32 -> YT [106, N_SENT] f32."""
    _ensure_ntff_hook()
    from concourse.bass_utils import run_bass_kernel_spmd

    variant = variant or VARIANT
    nc, xsplit = _build(variant)

    if variant == "f16":
        XF = XT.astype(np.float16)
        wpack = _pack_weights(CWT, np.float16)
        in_maps = [
            {
                "xf": np.ascontiguousarray(
                    XF[:, c * N_PER_CORE : (c + 1) * N_PER_CORE]
                ),
                "cwt": wpack,
            }
            for c in range(N_CORES)
        ]
    elif variant == "f32r":
        wpack = _pack_weights(CWT)
        in_maps = [
            {
                "xt": np.ascontiguousarray(
                    XT[:, c * N_PER_CORE : (c + 1) * N_PER_CORE]
                ),
                "cwt": wpack,
            }
            for c in range(N_CORES)
        ]
    else:
        import ml_dtypes

        bf16 = ml_dtypes.bfloat16
        XH = XT.astype(bf16)
        XL = (XT - XH.astype(np.float32)).astype(bf16)
        CWH = CWT.astype(np.float32).astype(bf16).astype(np.float32)
        CWL = CWT.astype(np.float32) - CWH
        wh = _pack_weights(CWH, bf16)
        wl = _pack_weights(CWL, bf16)
        in_maps = [
            {
                "xh": np.ascontiguousarray(
                    XH[:, c * N_PER_CORE : (c + 1) * N_PER_CORE]
                ),
                "xl": np.ascontiguousarray(
                    XL[:, c * N_PER_CORE : (c + 1) * N_PER_CORE]
                ),
                "cwh": wh,
                "cwl": wl,
            }
            for c in range(N_CORES)
        ]

    res = run_bass_kernel_spmd(nc, in_maps, list(range(N_CORES)), trace=trace)
    return _unpack_yt(res, xsplit, variant), res


def kernel(X, Constraints, W, b, X_Scope, X_Rel, _trace=False, _res_out=None):
    X = np.asarray(X)
    Constraints = np.asarray(Constraints)
    W = np.asarray(W)
    b = np.asarray(b)
    X_Scope = np.asarray(X_Scope)
    X_Rel = np.asarray(X_Rel)

    N, D = X.shape
    B = X_Scope.shape[0]
    R = Constraints.shape[0]
    assert (N, D, R) == (N_SENT, D_FEAT, N_REL), (N, D, R)

    XT = np.ascontiguousarray(X.T)
    CWT = np.ascontiguousarray(
        np.concatenate([Constraints, W], axis=0).T.astype(np.float32)
    )

    YT, res = _run_device(XT, CWT, trace=_trace)
    if _res_out is not None:
        _res_out.append(res)

    S_all = YT[:N_REL]          # [53, N] scores for every relation
    P = YT[N_REL:]              # [53, N] per-sentence classifier projections

    # host downstream on [N, 53]-sized data (mirrors reference semantics)
    starts = X_Scope[:, 0].astype(np.int64)
    seg = np.searchsorted(starts, np.arange(N, dtype=np.int64), side="right") - 1
    rel = np.asarray(X_Rel)[seg]  # wraps for seg == -1, same as jnp
    s = S_all[rel, np.arange(N)].astype(np.float64)

    valid = seg >= 0
    segv = seg[valid]
    m = np.full(B, -np.inf)
    np.maximum.at(m, segv, s[valid])
    e = np.exp(s - np.where(valid, m[np.clip(seg, 0, B - 1)], np.inf))
    e = np.where(valid, e, 0.0)
    z = np.bincount(segv, weights=e[valid], minlength=B)
    zsafe = np.where(z == 0.0, 1.0, z)
    w = e / zsafe[np.clip(seg, 0, B - 1)]

    out = np.empty((B, N_REL), dtype=np.float64)
    Pw = P.astype(np.float64) * w[None, :]
    for j in range(N_REL):
        out[:, j] = np.bincount(segv, weights=Pw[j, valid], minlength=B)
    out += b.astype(np.float64)[None, :]
    return out.astype(np.float32)

